# revision 8
# baseline (speedup 1.0000x reference)
"""TRN2 Bass kernel for nn_AlgebraicBlock (dense transformer block):
MR-norm -> QKV -> ALiBi attention w/ rational softmax -> out-proj residual ->
MR-norm -> rational SwiGLU FFN -> residual.   x: [1, 2048, 1024] f32.

Tensor-parallel over 8 NeuronCores:
  - heads split 2/core for attention (QKV col-split, ALiBi via 4 augmented
    contraction dims, flash-style late softmax normalization via a ones
    column in the PV matmul)
  - AllGather(attn) -> column-split out-projection (exact f32 residual shard)
  - tiny AllReduce for MR-norm column sums (feature dim is sharded)
  - AllGather(h2) -> column-split FFN1 -> row-split FFN2 -> ReduceScatter
All GEMMs bf16 with f32 PSUM accumulation; transposed [feature, T] layout
throughout so all row-softmax/norm reductions happen on the free dim or
through the PE.
"""

import os
import numpy as np
import ml_dtypes

T, C, H, D, F = 2048, 1024, 16, 64, 4096
NCORES = 8
EPS = 1e-6
P = 128
TT = T // 512          # 4 t-tiles of 512
CS = C // P            # 8 c-subtiles
BF = ml_dtypes.bfloat16

TRACE = False          # set True by test.py for neuron-profile timing
LAST_RESULTS = None    # BassKernelResults of the last run (for test.py)

_PROGRAM = None


def _bf16(x):
    return np.asarray(x, dtype=BF)


def _alibi_slopes():
    start = 2.0 ** (-8.0 / H)
    return (start ** np.arange(1, H + 1)).astype(np.float64)


def _prepare_in_maps(x, w_qkv, w_out, w_merged, w3, norm1_w, norm2_w):
    """Host-side sharding + weight preprocessing (layout/precision only)."""
    x = np.asarray(x, np.float32)[0]            # [T, C]
    xT = np.ascontiguousarray(x.T)              # [C, T]
    xt_bf = _bf16(xT)
    slopes = _alibi_slopes()

    pos = np.arange(T, dtype=np.float64)
    t_hi, t_lo = pos // 64, pos % 64

    Wn = np.asarray(w_qkv, np.float32) * np.asarray(norm1_w, np.float32)[None, :]
    w_out = np.asarray(w_out, np.float32)
    wm_n = np.asarray(w_merged, np.float32) * np.asarray(norm2_w, np.float32)[None, :]
    w3 = np.asarray(w3, np.float32)

    in_maps = []
    for i in range(NCORES):
        h0, h1 = 2 * i, 2 * i + 1
        rows = []
        for h in (h0, h1):
            rows.append(Wn[64 * h:64 * h + 64, :] * 0.125)          # q (scaled)
        for h in (h0, h1):
            rows.append(Wn[C + 64 * h:C + 64 * h + 64, :])          # k
        for h in (h0, h1):
            rows.append(Wn[2 * C + 64 * h:2 * C + 64 * h + 64, :])  # v
        wqkv_t = _bf16(np.ascontiguousarray(np.concatenate(rows, 0).T))  # [1024, 384]

        wout_t = _bf16(np.ascontiguousarray(w_out[P * i:P * (i + 1), :].T))  # [1024, 128]

        gsl = slice(512 * i, 512 * (i + 1))
        wm = np.concatenate([wm_n[gsl, :], wm_n[F:][gsl, :]], 0)     # [1024, C]
        wm_t = _bf16(np.ascontiguousarray(wm.T))                     # [1024, 1024]
        w3_t = _bf16(np.ascontiguousarray(w3[:, gsl].T))             # [512, 1024]

        aug_q = np.zeros((2, 4, T), np.float64)
        aug_k = np.zeros((2, 4, T), np.float64)
        for j, h in enumerate((h0, h1)):
            sl = float(_bf16(slopes[h]))
            aug_q[j, 0] = -t_hi
            aug_q[j, 1] = -t_lo
            aug_q[j, 2] = sl * 64
            aug_q[j, 3] = sl
            aug_k[j, 0] = sl * 64
            aug_k[j, 1] = sl
            aug_k[j, 2] = t_hi
            aug_k[j, 3] = t_lo

        maskt = np.zeros((4, P, 512), np.float64)
        for r in range(4):
            s_idx = P * r + np.arange(P)[:, None]
            maskt[r] = (s_idx <= np.arange(512)[None, :]).astype(np.float64)

        in_maps.append({
            "xt_bf": xt_bf,
            "xt_sh": np.ascontiguousarray(xT[P * i:P * (i + 1)]),
            "wqkv_t": wqkv_t,
            "wout_t": wout_t,
            "wm_t": wm_t,
            "w3_t": w3_t,
            "aug_q": _bf16(aug_q),
            "aug_k": _bf16(aug_k),
            "maskt": _bf16(maskt),
        })
    return in_maps


def _build_program():
    import concourse.bass as bass
    import concourse.mybir as mybir
    import concourse.tile as tile
    from concourse import bacc
    from concourse.masks import make_identity

    dt = mybir.dt
    Alu = mybir.AluOpType
    Act = mybir.ActivationFunctionType

    nc = bacc.Bacc("TRN2", target_bir_lowering=False, debug=False,
                   enable_asserts=True, num_devices=NCORES)

    # I/O
    xt_bf = nc.dram_tensor("xt_bf", [C, T], dt.bfloat16, kind="ExternalInput")
    xt_sh = nc.dram_tensor("xt_sh", [P, T], dt.float32, kind="ExternalInput")
    wqkv_t = nc.dram_tensor("wqkv_t", [C, 384], dt.bfloat16, kind="ExternalInput")
    wout_t = nc.dram_tensor("wout_t", [C, P], dt.bfloat16, kind="ExternalInput")
    wm_t = nc.dram_tensor("wm_t", [C, 1024], dt.bfloat16, kind="ExternalInput")
    w3_t = nc.dram_tensor("w3_t", [512, 1024], dt.bfloat16, kind="ExternalInput")
    aug_q = nc.dram_tensor("aug_q", [2, 4, T], dt.bfloat16, kind="ExternalInput")
    aug_k = nc.dram_tensor("aug_k", [2, 4, T], dt.bfloat16, kind="ExternalInput")
    maskt = nc.dram_tensor("maskt", [4, P, 512], dt.bfloat16, kind="ExternalInput")
    out = nc.dram_tensor("out", [P, T], dt.float32, kind="ExternalOutput")

    # internal DRAM (collective bounces)
    cs1_in = nc.dram_tensor("cs1_in", [1, T], dt.float32)
    cs1_out = nc.dram_tensor("cs1_out", [1, T], dt.float32, addr_space="Shared")
    attn_in = nc.dram_tensor("attn_in", [P, T], dt.bfloat16)
    attn_out = nc.dram_tensor("attn_out", [C, T], dt.bfloat16, addr_space="Shared")
    cs2_in = nc.dram_tensor("cs2_in", [1, T], dt.float32)
    cs2_out = nc.dram_tensor("cs2_out", [1, T], dt.float32, addr_space="Shared")
    h2_in = nc.dram_tensor("h2_in", [P, T], dt.bfloat16)
    h2_out = nc.dram_tensor("h2_out", [C, T], dt.bfloat16, addr_space="Shared")
    rs_in = nc.dram_tensor("rs_in", [C, T], dt.bfloat16)
    rs_out = nc.dram_tensor("rs_out", [P, T], dt.bfloat16)

    RG = [list(range(NCORES))]

    with tile.TileContext(nc, num_cores=NCORES) as tc:
        with (
            tc.tile_pool(name="wpool", bufs=1) as wpool,
            tc.tile_pool(name="big", bufs=1) as big,
            tc.tile_pool(name="mid", bufs=1) as mid,
            tc.tile_pool(name="work", bufs=2) as work,
            tc.tile_pool(name="tiny", bufs=1) as tiny,
            tc.tile_pool(name="tiny2", bufs=2) as tiny2,
            tc.tile_pool(name="psmm", bufs=5, space="PSUM") as psmm,
            tc.tile_pool(name="pspv", bufs=2, space="PSUM") as pspv,
        ):
            # ---------- constants / weights ----------
            ones_sb = wpool.tile([P, 1], dt.bfloat16)
            nc.vector.memset(ones_sb[:], 1.0)
            ident = wpool.tile([P, P], dt.bfloat16)
            make_identity(nc, ident[:])

            wqkv_sb = wpool.tile([P, CS, 384], dt.bfloat16)
            nc.sync.dma_start(wqkv_sb[:], wqkv_t.ap().rearrange("(o p) m -> p o m", p=P))
            wout_sb = wpool.tile([P, CS, P], dt.bfloat16)
            nc.sync.dma_start(wout_sb[:], wout_t.ap().rearrange("(o p) m -> p o m", p=P))
            wm_sb = wpool.tile([P, CS, 1024], dt.bfloat16)
            nc.sync.dma_start(wm_sb[:], wm_t.ap().rearrange("(o p) m -> p o m", p=P))
            w3_sb = wpool.tile([P, 4, 1024], dt.bfloat16)
            nc.sync.dma_start(w3_sb[:], w3_t.ap().rearrange("(o p) m -> p o m", p=P))
            mask_sb = wpool.tile([P, 4, 512], dt.bfloat16)
            nc.sync.dma_start(mask_sb[:], maskt.ap().rearrange("r p f -> p r f"))

            xt_sb = big.tile([P, CS, T], dt.bfloat16, tag="big3")
            nc.sync.dma_start(xt_sb[:], xt_bf.ap().rearrange("(o p) t -> p o t", p=P))

            # ---------- P1: invmean1 (local c-shard colsum + AllReduce) ----------
            # this core's shard rows of xT live in xt_sh (f32) — use the bf16
            # copy rows instead: they are a contiguous [P, T] slice of xt_bf on
            # a per-core basis only via data; use xt_sh (f32) and abs it.
            xtsh_sb = mid.tile([P, T], dt.float32, tag="xtsh")
            nc.sync.dma_start(xtsh_sb[:], xt_sh.ap())
            absx = mid.tile([P, T], dt.bfloat16, tag="absx")
            nc.scalar.activation(absx[:], xtsh_sb[:], Act.Abs)
            cs_sb = tiny.tile([1, T], dt.float32, tag="cs")
            for t4 in range(TT):
                csp = pspv.tile([1, 512], dt.float32, tag="pv")
                nc.tensor.matmul(csp[:], ones_sb[:], absx[:, 512 * t4:512 * (t4 + 1)],
                                 start=True, stop=True)
                nc.scalar.copy(cs_sb[:, 512 * t4:512 * (t4 + 1)], csp[:])
            nc.sync.dma_start(cs1_in[:], cs_sb[:])
            nc.gpsimd.collective_compute(
                "AllReduce", Alu.add, replica_groups=RG,
                ins=[cs1_in[:]], outs=[cs1_out[:]])
            s1_sb = tiny.tile([1, T], dt.float32, tag="s1")
            nc.sync.dma_start(s1_sb[:], cs1_out[:])
            nc.vector.tensor_scalar(s1_sb[:], s1_sb[:], 1.0 / C, EPS, Alu.mult, Alu.add)
            r1_sb = tiny.tile([1, T], dt.float32, tag="r1")
            nc.vector.reciprocal_approx_fast(r1_sb[:], s1_sb[:])
            r1b_sb = tiny.tile([1, T], dt.bfloat16, tag="r1b")
            nc.scalar.copy(r1b_sb[:], r1_sb[:])
            inv1_sb = mid.tile([P, T], dt.bfloat16, tag="inv1")
            nc.gpsimd.partition_broadcast(inv1_sb[:], r1b_sb[:])

            # ---------- P3: QKV GEMM + v transpose ----------
            qa = [mid.tile([P, T], dt.bfloat16, tag=f"qa{j}", name=f"qa{j}")
                  for j in range(2)]
            ka = [mid.tile([P, T], dt.bfloat16, tag=f"ka{j}", name=f"ka{j}")
                  for j in range(2)]
            v_sb = [mid.tile([P, 16, 65], dt.bfloat16, tag=f"v{j}", name=f"v{j}")
                    for j in range(2)]
            for j in range(2):
                nc.vector.memset(qa[j][64:128, :], 0.0)
                nc.vector.memset(ka[j][64:128, :], 0.0)
                nc.sync.dma_start(qa[j][64:68, :], aug_q.ap()[j])
                nc.sync.dma_start(ka[j][64:68, :], aug_k.ap()[j])
                nc.vector.memset(v_sb[j][:, :, 64:65], 1.0)

            for ch in range(3):
                for t4 in range(TT):
                    tsl = slice(512 * t4, 512 * (t4 + 1))
                    ps = psmm.tile([P, 512], dt.float32, tag="mm")
                    for o in range(CS):
                        nc.tensor.matmul(ps[:], wqkv_sb[:, o, 128 * ch:128 * (ch + 1)],
                                         xt_sb[:, o, tsl], start=(o == 0), stop=(o == CS - 1))
                    if ch == 0:
                        for j in range(2):
                            nc.vector.scalar_tensor_tensor(
                                qa[j][0:64, tsl], ps[64 * j:64 * j + 64, :], 0.0,
                                inv1_sb[0:64, tsl], Alu.bypass, Alu.mult)
                    elif ch == 1:
                        for j in range(2):
                            nc.vector.scalar_tensor_tensor(
                                ka[j][0:64, tsl], ps[64 * j:64 * j + 64, :], 0.0,
                                inv1_sb[0:64, tsl], Alu.bypass, Alu.mult)
                    else:
                        vt_w = work.tile([P, 512], dt.bfloat16, tag="vtw")
                        nc.vector.scalar_tensor_tensor(
                            vt_w[:], ps[:], 0.0, inv1_sb[:, tsl], Alu.bypass, Alu.mult)
                        for u in range(4):
                            st = 4 * t4 + u
                            tp = psmm.tile([P, P], dt.bfloat16, tag="mm")
                            nc.tensor.transpose(tp[:], vt_w[:, P * u:P * (u + 1)], ident[:])
                            for j in range(2):
                                nc.scalar.copy(v_sb[j][:, st, 0:64],
                                               tp[:, 64 * j:64 * j + 64])

            # ---------- P4: attention ----------
            attn_sb = mid.tile([P, T], dt.bfloat16, tag="attn")
            for j in range(2):
                for t4 in range(TT):
                    tsl = slice(512 * t4, 512 * (t4 + 1))
                    nst = 4 * t4 + 4
                    pv = pspv.tile([65, 512], dt.float32, tag="pv")
                    for k in range(nst):
                        sp = psmm.tile([P, 512], dt.float32, tag="mm")
                        nc.tensor.matmul(sp[:], ka[j][:, P * k:P * (k + 1)],
                                         qa[j][:, tsl], start=True, stop=True)
                        a = work.tile([P, 512], dt.float32, tag="a")
                        nc.scalar.activation(a[:], sp[:], Act.Abs)
                        b = work.tile([P, 512], dt.float32, tag="b")
                        nc.gpsimd.tensor_scalar(b[:], a[:], 1.0, None, Alu.add)
                        r = work.tile([P, 512], dt.float32, tag="r")
                        nc.vector.reciprocal_approx_fast(r[:], b[:])
                        sr = work.tile([P, 512], dt.bfloat16, tag="sr")
                        nc.vector.scalar_tensor_tensor(sr[:], sp[:], 1.0, r[:],
                                                       Alu.mult, Alu.mult)
                        u2 = work.tile([P, 512], dt.bfloat16, tag="u2")
                        nc.scalar.activation(u2[:], sr[:], Act.Square, bias=1.0, scale=1.0)
                        if k >= 4 * t4:
                            nc.vector.tensor_tensor(u2[:], u2[:],
                                                    mask_sb[:, k - 4 * t4, :], Alu.mult)
                        p4t = work.tile([P, 512], dt.bfloat16, tag="p4")
                        nc.vector.tensor_tensor(p4t[:], u2[:], u2[:], Alu.mult)
                        nc.tensor.matmul(pv[:], v_sb[j][:, k, :], p4t[:],
                                         start=(k == 0), stop=(k == nst - 1))
                    de = tiny2.tile([1, 512], dt.float32, tag="de")
                    nc.vector.tensor_scalar(de[:], pv[64:65, :], 16.0 * EPS, None, Alu.add)
                    rd = tiny2.tile([1, 512], dt.float32, tag="rd")
                    nc.vector.reciprocal_approx_fast(rd[:], de[:])
                    rdb = tiny2.tile([1, 512], dt.bfloat16, tag="rdb")
                    nc.scalar.copy(rdb[:], rd[:])
                    rdbb = work.tile([64, 512], dt.bfloat16, tag="rdbb")
                    nc.gpsimd.partition_broadcast(rdbb[:], rdb[:])
                    nc.vector.tensor_tensor(attn_sb[64 * j:64 * j + 64, tsl],
                                            pv[0:64, :], rdbb[:], Alu.mult)

            # ---------- P5: AllGather(attn) ----------
            nc.sync.dma_start(attn_in[:], attn_sb[:])
            nc.gpsimd.collective_compute(
                "AllGather", Alu.bypass, replica_groups=RG,
                ins=[attn_in[:]], outs=[attn_out[:]])
            af_sb = big.tile([P, CS, T], dt.bfloat16, tag="big3")
            nc.sync.dma_start(af_sb[:], attn_out.ap().rearrange("(o p) t -> p o t", p=P))

            # ---------- P6: out-proj (col-split) + residual + norm2 ----------
            x2_sb = mid.tile([P, T], dt.float32, tag="x2")
            for t4 in range(TT):
                tsl = slice(512 * t4, 512 * (t4 + 1))
                yp = psmm.tile([P, 512], dt.float32, tag="mm")
                for o in range(CS):
                    nc.tensor.matmul(yp[:], wout_sb[:, o, :], af_sb[:, o, tsl],
                                     start=(o == 0), stop=(o == CS - 1))
                nc.vector.tensor_tensor(x2_sb[:, tsl], yp[:], xtsh_sb[:, tsl], Alu.add)
            abs2 = mid.tile([P, T], dt.bfloat16, tag="absx")
            nc.scalar.activation(abs2[:], x2_sb[:], Act.Abs)
            cs2_sb = tiny.tile([1, T], dt.float32, tag="cs")
            for t4 in range(TT):
                csp = pspv.tile([1, 512], dt.float32, tag="pv")
                nc.tensor.matmul(csp[:], ones_sb[:], abs2[:, 512 * t4:512 * (t4 + 1)],
                                 start=True, stop=True)
                nc.scalar.copy(cs2_sb[:, 512 * t4:512 * (t4 + 1)], csp[:])
            nc.sync.dma_start(cs2_in[:], cs2_sb[:])
            nc.gpsimd.collective_compute(
                "AllReduce", Alu.add, replica_groups=RG,
                ins=[cs2_in[:]], outs=[cs2_out[:]])
            s2_sb = tiny.tile([1, T], dt.float32, tag="s1")
            nc.sync.dma_start(s2_sb[:], cs2_out[:])
            nc.vector.tensor_scalar(s2_sb[:], s2_sb[:], 1.0 / C, EPS, Alu.mult, Alu.add)
            r2_sb = tiny.tile([1, T], dt.float32, tag="r1")
            nc.vector.reciprocal_approx_fast(r2_sb[:], s2_sb[:])
            r2b_sb = tiny.tile([1, T], dt.bfloat16, tag="r1b")
            nc.scalar.copy(r2b_sb[:], r2_sb[:])
            inv2_sb = mid.tile([P, T], dt.bfloat16, tag="inv1")
            nc.gpsimd.partition_broadcast(inv2_sb[:], r2b_sb[:])
            h2sh_sb = mid.tile([P, T], dt.bfloat16, tag="attn")
            nc.vector.tensor_tensor(h2sh_sb[:], x2_sb[:], inv2_sb[:], Alu.mult)

            # ---------- AllGather(h2) ----------
            nc.sync.dma_start(h2_in[:], h2sh_sb[:])
            nc.gpsimd.collective_compute(
                "AllGather", Alu.bypass, replica_groups=RG,
                ins=[h2_in[:]], outs=[h2_out[:]])
            h2f_sb = big.tile([P, CS, T], dt.bfloat16, tag="big3")
            nc.sync.dma_start(h2f_sb[:], h2_out.ap().rearrange("(o p) t -> p o t", p=P))

            # ---------- P7: FFN1 + rational SwiGLU ----------
            hidT = mid.tile([P, 4, T], dt.bfloat16, tag="hid")
            for fc in range(4):
                for t4 in range(TT):
                    tsl = slice(512 * t4, 512 * (t4 + 1))
                    gp = psmm.tile([P, 512], dt.float32, tag="mm")
                    vp = psmm.tile([P, 512], dt.float32, tag="mm")
                    for o in range(CS):
                        nc.tensor.matmul(gp[:], wm_sb[:, o, 128 * fc:128 * (fc + 1)],
                                         h2f_sb[:, o, tsl], start=(o == 0), stop=(o == CS - 1))
                    for o in range(CS):
                        nc.tensor.matmul(vp[:], wm_sb[:, o, 512 + 128 * fc:512 + 128 * (fc + 1)],
                                         h2f_sb[:, o, tsl], start=(o == 0), stop=(o == CS - 1))
                    a2 = work.tile([P, 512], dt.float32, tag="a")
                    nc.scalar.activation(a2[:], gp[:], Act.Abs)
                    b2 = work.tile([P, 512], dt.float32, tag="b")
                    nc.gpsimd.tensor_scalar(b2[:], a2[:], 1.0, None, Alu.add)
                    rr = work.tile([P, 512], dt.float32, tag="r")
                    nc.vector.reciprocal_approx_fast(rr[:], b2[:])
                    nn = work.tile([P, 512], dt.float32, tag="nn")
                    nc.vector.tensor_scalar(nn[:], gp[:], 0.0, 0.5, Alu.max, Alu.add)
                    nr = work.tile([P, 512], dt.bfloat16, tag="sr")
                    nc.vector.tensor_tensor(nr[:], nn[:], rr[:], Alu.mult)
                    vs = work.tile([P, 512], dt.bfloat16, tag="u2")
                    nc.scalar.copy(vs[:], vp[:])
                    gv = work.tile([P, 512], dt.bfloat16, tag="p4")
                    nc.vector.tensor_tensor(gv[:], gp[:], vs[:], Alu.mult)
                    nc.vector.tensor_tensor(hidT[:, fc, tsl], nr[:], gv[:], Alu.mult)

            # ---------- P8: FFN2 (row-split) -> ReduceScatter ----------
            for jc in range(CS):
                for t4 in range(TT):
                    tsl = slice(512 * t4, 512 * (t4 + 1))
                    zp = psmm.tile([P, 512], dt.float32, tag="mm")
                    for o in range(4):
                        nc.tensor.matmul(zp[:], w3_sb[:, o, 128 * jc:128 * (jc + 1)],
                                         hidT[:, o, tsl], start=(o == 0), stop=(o == 3))
                    zs = work.tile([P, 512], dt.bfloat16, tag="zs")
                    nc.scalar.copy(zs[:], zp[:])
                    nc.sync.dma_start(rs_in[P * jc:P * (jc + 1), tsl], zs[:])
            nc.gpsimd.collective_compute(
                "ReduceScatter", Alu.add, replica_groups=RG,
                ins=[rs_in[:]], outs=[rs_out[:]])

            # ---------- P9: final residual ----------
            rso_sb = mid.tile([P, T], dt.bfloat16, tag="absx")
            nc.sync.dma_start(rso_sb[:], rs_out[:])
            o_sb = mid.tile([P, T], dt.float32, tag="xtsh")
            nc.vector.tensor_tensor(o_sb[:], rso_sb[:], x2_sb[:], Alu.add)
            nc.sync.dma_start(out[:], o_sb[:])

    nc.compile()
    return nc


def _get_program():
    global _PROGRAM
    if _PROGRAM is None:
        _PROGRAM = _build_program()
    return _PROGRAM


def kernel(x, w_qkv, w_out, w_merged, w3, norm1_w, norm2_w):
    global LAST_RESULTS
    from concourse.bass_utils import run_bass_kernel_spmd

    nc = _get_program()
    in_maps = _prepare_in_maps(x, w_qkv, w_out, w_merged, w3, norm1_w, norm2_w)
    res = run_bass_kernel_spmd(nc, in_maps, core_ids=list(range(NCORES)),
                               trace=TRACE)
    LAST_RESULTS = res
    yT = np.concatenate([res.results[i]["out"] for i in range(NCORES)], axis=0)
    return np.ascontiguousarray(yT.T)[None].astype(np.float32)


# revision 9
# speedup vs baseline: 1.7176x; 1.7176x over previous
"""TRN2 Bass kernel for nn_AlgebraicBlock (dense transformer block):
MR-norm -> QKV -> ALiBi attention w/ rational softmax -> out-proj residual ->
MR-norm -> rational SwiGLU FFN -> residual.   x: [1, 2048, 1024] f32.

Tensor-parallel over 8 NeuronCores:
  - heads split 2/core for attention (QKV col-split, ALiBi via 4 augmented
    contraction dims, flash-style late softmax normalization via a ones
    column in the PV matmul)
  - AllGather(attn) -> column-split out-projection (exact f32 residual shard)
  - tiny AllReduce for MR-norm column sums (feature dim is sharded)
  - AllGather(h2) -> column-split FFN1 -> row-split FFN2 -> ReduceScatter
All GEMMs bf16 with f32 PSUM accumulation; transposed [feature, T] layout
throughout so all row-softmax/norm reductions happen on the free dim or
through the PE.
"""

import os
import numpy as np
import ml_dtypes

T, C, H, D, F = 2048, 1024, 16, 64, 4096
NCORES = 8
EPS = 1e-6
P = 128
TT = T // 512          # 4 t-tiles of 512
CS = C // P            # 8 c-subtiles
BF = ml_dtypes.bfloat16

TRACE = False          # set True by test.py for neuron-profile timing
LAST_RESULTS = None    # BassKernelResults of the last run (for test.py)

_PROGRAM = None


def _bf16(x):
    return np.asarray(x, dtype=BF)


def _alibi_slopes():
    start = 2.0 ** (-8.0 / H)
    return (start ** np.arange(1, H + 1)).astype(np.float64)


def _prepare_in_maps(x, w_qkv, w_out, w_merged, w3, norm1_w, norm2_w):
    """Host-side sharding + weight preprocessing (layout/precision only)."""
    x = np.asarray(x, np.float32)[0]            # [T, C]
    xT = np.ascontiguousarray(x.T)              # [C, T]
    xt_bf = _bf16(xT)
    slopes = _alibi_slopes()

    pos = np.arange(T, dtype=np.float64)
    t_hi, t_lo = pos // 64, pos % 64

    Wn = np.asarray(w_qkv, np.float32) * np.asarray(norm1_w, np.float32)[None, :]
    w_out = np.asarray(w_out, np.float32)
    wm_n = np.asarray(w_merged, np.float32) * np.asarray(norm2_w, np.float32)[None, :]
    w3 = np.asarray(w3, np.float32)

    in_maps = []
    for i in range(NCORES):
        h0, h1 = 2 * i, 2 * i + 1
        rows = []
        for h in (h0, h1):
            rows.append(Wn[64 * h:64 * h + 64, :] * 0.125)          # q (scaled)
        for h in (h0, h1):
            rows.append(Wn[C + 64 * h:C + 64 * h + 64, :])          # k
        for h in (h0, h1):
            rows.append(Wn[2 * C + 64 * h:2 * C + 64 * h + 64, :])  # v
        wqkv_t = _bf16(np.ascontiguousarray(np.concatenate(rows, 0).T))  # [1024, 384]

        wout_t = _bf16(np.ascontiguousarray(w_out[P * i:P * (i + 1), :].T))  # [1024, 128]

        gsl = slice(512 * i, 512 * (i + 1))
        wm = np.concatenate([wm_n[gsl, :], wm_n[F:][gsl, :]], 0)     # [1024, C]
        wm_t = _bf16(np.ascontiguousarray(wm.T))                     # [1024, 1024]
        w3_t = _bf16(np.ascontiguousarray(w3[:, gsl].T))             # [512, 1024]

        aug_q = np.zeros((2, 4, T), np.float64)
        aug_k = np.zeros((2, 4, T), np.float64)
        for j, h in enumerate((h0, h1)):
            sl = float(_bf16(slopes[h]))
            aug_q[j, 0] = -t_hi
            aug_q[j, 1] = -t_lo
            aug_q[j, 2] = sl * 64
            aug_q[j, 3] = sl
            aug_k[j, 0] = sl * 64
            aug_k[j, 1] = sl
            aug_k[j, 2] = t_hi
            aug_k[j, 3] = t_lo

        maskt = np.zeros((4, P, 512), np.float64)
        for r in range(4):
            s_idx = P * r + np.arange(P)[:, None]
            maskt[r] = (s_idx <= np.arange(512)[None, :]).astype(np.float64)

        in_maps.append({
            "xt_bf": xt_bf,
            "xt_sh": np.ascontiguousarray(xT[P * i:P * (i + 1)]),
            "wqkv_t": wqkv_t,
            "wout_t": wout_t,
            "wm_t": wm_t,
            "w3_t": w3_t,
            "aug_q": _bf16(aug_q),
            "aug_k": _bf16(aug_k),
            "maskt": _bf16(maskt),
        })
    return in_maps


def _build_program():
    import concourse.bass as bass
    import concourse.mybir as mybir
    import concourse.tile as tile
    from concourse import bacc
    from concourse.masks import make_identity

    dt = mybir.dt
    Alu = mybir.AluOpType
    Act = mybir.ActivationFunctionType

    nc = bacc.Bacc("TRN2", target_bir_lowering=False, debug=False,
                   enable_asserts=True, num_devices=NCORES)

    # I/O
    xt_bf = nc.dram_tensor("xt_bf", [C, T], dt.bfloat16, kind="ExternalInput")
    xt_sh = nc.dram_tensor("xt_sh", [P, T], dt.float32, kind="ExternalInput")
    wqkv_t = nc.dram_tensor("wqkv_t", [C, 384], dt.bfloat16, kind="ExternalInput")
    wout_t = nc.dram_tensor("wout_t", [C, P], dt.bfloat16, kind="ExternalInput")
    wm_t = nc.dram_tensor("wm_t", [C, 1024], dt.bfloat16, kind="ExternalInput")
    w3_t = nc.dram_tensor("w3_t", [512, 1024], dt.bfloat16, kind="ExternalInput")
    aug_q = nc.dram_tensor("aug_q", [2, 4, T], dt.bfloat16, kind="ExternalInput")
    aug_k = nc.dram_tensor("aug_k", [2, 4, T], dt.bfloat16, kind="ExternalInput")
    maskt = nc.dram_tensor("maskt", [4, P, 512], dt.bfloat16, kind="ExternalInput")
    out = nc.dram_tensor("out", [P, T], dt.float32, kind="ExternalOutput")

    # internal DRAM (collective bounces)
    cs1_in = nc.dram_tensor("cs1_in", [1, T], dt.float32)
    cs1_out = nc.dram_tensor("cs1_out", [1, T], dt.float32, addr_space="Shared")
    attn_in = nc.dram_tensor("attn_in", [P, T], dt.bfloat16)
    attn_out = nc.dram_tensor("attn_out", [C, T], dt.bfloat16, addr_space="Shared")
    cs2_in = nc.dram_tensor("cs2_in", [1, T], dt.float32)
    cs2_out = nc.dram_tensor("cs2_out", [1, T], dt.float32, addr_space="Shared")
    h2_in = nc.dram_tensor("h2_in", [P, T], dt.bfloat16)
    h2_out = nc.dram_tensor("h2_out", [C, T], dt.bfloat16, addr_space="Shared")
    rs_in = nc.dram_tensor("rs_in", [C, T], dt.bfloat16)
    rs_out = nc.dram_tensor("rs_out", [P, T], dt.bfloat16)

    RG = [list(range(NCORES))]

    with tile.TileContext(nc, num_cores=NCORES) as tc:
        with (
            tc.tile_pool(name="wpool", bufs=1) as wpool,
            tc.tile_pool(name="big", bufs=1) as big,
            tc.tile_pool(name="mid", bufs=1) as mid,
            tc.tile_pool(name="workA", bufs=2) as workA,
            tc.tile_pool(name="workB", bufs=3) as workB,
            tc.tile_pool(name="workC", bufs=2) as workC,
            tc.tile_pool(name="tiny", bufs=1) as tiny,
            tc.tile_pool(name="tiny2", bufs=2) as tiny2,
            tc.tile_pool(name="psmm", bufs=5, space="PSUM") as psmm,
            tc.tile_pool(name="pspv", bufs=2, space="PSUM") as pspv,
        ):
            # ---------- constants / weights ----------
            ones_sb = wpool.tile([P, 1], dt.bfloat16)
            nc.vector.memset(ones_sb[:], 1.0)
            ident = wpool.tile([P, P], dt.bfloat16)
            make_identity(nc, ident[:])

            wqkv_sb = mid.tile([P, CS, 384], dt.bfloat16, tag="hid", name="wqkv_sb")
            nc.sync.dma_start(wqkv_sb[:], wqkv_t.ap().rearrange("(o p) m -> p o m", p=P))
            wout_sb = wpool.tile([P, CS, P], dt.bfloat16)
            nc.sync.dma_start(wout_sb[:], wout_t.ap().rearrange("(o p) m -> p o m", p=P))
            wm_sb = wpool.tile([P, CS, 1024], dt.bfloat16)
            nc.sync.dma_start(wm_sb[:], wm_t.ap().rearrange("(o p) m -> p o m", p=P))
            w3_sb = wpool.tile([P, 4, 1024], dt.bfloat16)
            nc.sync.dma_start(w3_sb[:], w3_t.ap().rearrange("(o p) m -> p o m", p=P))
            mask_sb = wpool.tile([P, 4, 512], dt.bfloat16)
            nc.sync.dma_start(mask_sb[:], maskt.ap().rearrange("r p f -> p r f"))

            xt_sb = big.tile([P, CS, T], dt.bfloat16, tag="big3")
            nc.sync.dma_start(xt_sb[:], xt_bf.ap().rearrange("(o p) t -> p o t", p=P))

            # ---------- P1: invmean1 (local c-shard colsum + AllReduce) ----------
            # this core's shard rows of xT live in xt_sh (f32) — use the bf16
            # copy rows instead: they are a contiguous [P, T] slice of xt_bf on
            # a per-core basis only via data; use xt_sh (f32) and abs it.
            cs_sb = tiny.tile([1, T], dt.float32, tag="cs")
            for t4 in range(TT):
                tsl = slice(512 * t4, 512 * (t4 + 1))
                xsl = workC.tile([P, 512], dt.float32, tag="xsl")
                nc.sync.dma_start(xsl[:], xt_sh.ap()[:, tsl])
                ax = workB.tile([P, 512], dt.bfloat16, tag="u2")
                nc.scalar.activation(ax[:], xsl[:], Act.Abs)
                csp = pspv.tile([1, 512], dt.float32, tag="pv")
                nc.tensor.matmul(csp[:], ones_sb[:], ax[:],
                                 start=True, stop=True)
                nc.scalar.copy(cs_sb[:, 512 * t4:512 * (t4 + 1)], csp[:])
            nc.sync.dma_start(cs1_in[:], cs_sb[:])
            nc.gpsimd.collective_compute(
                "AllReduce", Alu.add, replica_groups=RG,
                ins=[cs1_in[:]], outs=[cs1_out[:]])
            s1_sb = tiny.tile([1, T], dt.float32, tag="s1")
            nc.sync.dma_start(s1_sb[:], cs1_out[:])
            nc.vector.tensor_scalar(s1_sb[:], s1_sb[:], 1.0 / C, EPS, Alu.mult, Alu.add)
            r1_sb = tiny.tile([1, T], dt.float32, tag="r1")
            nc.vector.reciprocal_approx_fast(r1_sb[:], s1_sb[:])
            r1b_sb = tiny.tile([1, T], dt.bfloat16, tag="r1b")
            nc.scalar.copy(r1b_sb[:], r1_sb[:])
            inv1_sb = mid.tile([P, T], dt.bfloat16, tag="inv1")
            nc.gpsimd.partition_broadcast(inv1_sb[:], r1b_sb[:])

            # ---------- P3: QKV GEMM + v transpose ----------
            qa = [mid.tile([P, T], dt.bfloat16, tag=f"qa{j}", name=f"qa{j}")
                  for j in range(2)]
            ka = [mid.tile([P, T], dt.bfloat16, tag=f"ka{j}", name=f"ka{j}")
                  for j in range(2)]
            v_sb = [mid.tile([P, 16, 65], dt.bfloat16, tag=f"v{j}", name=f"v{j}")
                    for j in range(2)]
            for j in range(2):
                nc.vector.memset(qa[j][64:128, :], 0.0)
                nc.vector.memset(ka[j][64:128, :], 0.0)
                nc.sync.dma_start(qa[j][64:68, :], aug_q.ap()[j])
                nc.sync.dma_start(ka[j][64:68, :], aug_k.ap()[j])
                nc.vector.memset(v_sb[j][:, :, 64:65], 1.0)

            for ch in range(3):
                for t4 in range(TT):
                    tsl = slice(512 * t4, 512 * (t4 + 1))
                    ps = psmm.tile([P, 512], dt.float32, tag="mm")
                    for o in range(CS):
                        nc.tensor.matmul(ps[:], wqkv_sb[:, o, 128 * ch:128 * (ch + 1)],
                                         xt_sb[:, o, tsl], start=(o == 0), stop=(o == CS - 1))
                    if ch == 0:
                        for j in range(2):
                            nc.vector.scalar_tensor_tensor(
                                qa[j][0:64, tsl], ps[64 * j:64 * j + 64, :], 0.0,
                                inv1_sb[0:64, tsl], Alu.bypass, Alu.mult)
                    elif ch == 1:
                        for j in range(2):
                            nc.vector.scalar_tensor_tensor(
                                ka[j][0:64, tsl], ps[64 * j:64 * j + 64, :], 0.0,
                                inv1_sb[0:64, tsl], Alu.bypass, Alu.mult)
                    else:
                        vt_w = workC.tile([P, 512], dt.bfloat16, tag="vtw")
                        nc.vector.scalar_tensor_tensor(
                            vt_w[:], ps[:], 0.0, inv1_sb[:, tsl], Alu.bypass, Alu.mult)
                        for u in range(4):
                            st = 4 * t4 + u
                            tp = psmm.tile([P, P], dt.bfloat16, tag="mm")
                            nc.tensor.transpose(tp[:], vt_w[:, P * u:P * (u + 1)], ident[:])
                            for j in range(2):
                                nc.scalar.copy(v_sb[j][:, st, 0:64],
                                               tp[:, 64 * j:64 * j + 64])

            # ---------- P4: attention ----------
            attn_sb = mid.tile([P, T], dt.bfloat16, tag="attn")
            for j in range(2):
                for t4 in range(TT):
                    tsl = slice(512 * t4, 512 * (t4 + 1))
                    nst = 4 * t4 + 4
                    pv = pspv.tile([65, 512], dt.float32, tag="pv")
                    for k in range(nst):
                        sp = psmm.tile([P, 512], dt.float32, tag="mm")
                        nc.tensor.matmul(sp[:], ka[j][:, P * k:P * (k + 1)],
                                         qa[j][:, tsl], start=True, stop=True)
                        a = workA.tile([P, 512], dt.float32, tag="a")
                        nc.scalar.activation(a[:], sp[:], Act.Abs)
                        b = workA.tile([P, 512], dt.float32, tag="b")
                        nc.vector.tensor_scalar(b[:], a[:], 1.0, None, Alu.add)
                        r = workA.tile([P, 512], dt.float32, tag="r")
                        nc.vector.reciprocal_approx_fast(r[:], b[:])
                        sr = workB.tile([P, 512], dt.bfloat16, tag="sr")
                        nc.vector.scalar_tensor_tensor(sr[:], sp[:], 1.0, r[:],
                                                       Alu.mult, Alu.mult)
                        u2 = workB.tile([P, 512], dt.bfloat16, tag="u2")
                        nc.scalar.activation(u2[:], sr[:], Act.Square, bias=1.0, scale=1.0)
                        if k >= 4 * t4:
                            u2m = workB.tile([P, 512], dt.bfloat16, tag="sr")
                            nc.vector.tensor_tensor(u2m[:], u2[:],
                                                    mask_sb[:, k - 4 * t4, :], Alu.mult)
                            u2 = u2m
                        p4t = workB.tile([P, 512], dt.bfloat16, tag="p4")
                        nc.scalar.activation(p4t[:], u2[:], Act.Square)
                        nc.tensor.matmul(pv[:], v_sb[j][:, k, :], p4t[:],
                                         start=(k == 0), stop=(k == nst - 1))
                    de = tiny2.tile([1, 512], dt.float32, tag="de")
                    nc.vector.tensor_scalar(de[:], pv[64:65, :], 16.0 * EPS, None, Alu.add)
                    rd = tiny2.tile([1, 512], dt.float32, tag="rd")
                    nc.vector.reciprocal_approx_fast(rd[:], de[:])
                    rdb = tiny2.tile([1, 512], dt.bfloat16, tag="rdb")
                    nc.scalar.copy(rdb[:], rd[:])
                    rdbb = workC.tile([64, 512], dt.bfloat16, tag="rdbb")
                    nc.gpsimd.partition_broadcast(rdbb[:], rdb[:])
                    nc.vector.tensor_tensor(attn_sb[64 * j:64 * j + 64, tsl],
                                            pv[0:64, :], rdbb[:], Alu.mult)

            # ---------- P5: AllGather(attn) ----------
            nc.sync.dma_start(attn_in[:], attn_sb[:])
            nc.gpsimd.collective_compute(
                "AllGather", Alu.bypass, replica_groups=RG,
                ins=[attn_in[:]], outs=[attn_out[:]])
            af_sb = big.tile([P, CS, T], dt.bfloat16, tag="big3")
            nc.sync.dma_start(af_sb[:], attn_out.ap().rearrange("(o p) t -> p o t", p=P))

            # ---------- P6: out-proj (col-split) + residual + norm2 ----------
            x2_sb = mid.tile([P, T], dt.float32, tag="x2")
            for t4 in range(TT):
                tsl = slice(512 * t4, 512 * (t4 + 1))
                yp = psmm.tile([P, 512], dt.float32, tag="mm")
                for o in range(CS):
                    nc.tensor.matmul(yp[:], wout_sb[:, o, :], af_sb[:, o, tsl],
                                     start=(o == 0), stop=(o == CS - 1))
                xsl = workC.tile([P, 512], dt.float32, tag="xsl")
                nc.sync.dma_start(xsl[:], xt_sh.ap()[:, tsl])
                nc.vector.tensor_tensor(x2_sb[:, tsl], yp[:], xsl[:], Alu.add)
            cs2_sb = tiny.tile([1, T], dt.float32, tag="cs")
            for t4 in range(TT):
                tsl = slice(512 * t4, 512 * (t4 + 1))
                ax2 = workB.tile([P, 512], dt.bfloat16, tag="u2")
                nc.scalar.activation(ax2[:], x2_sb[:, tsl], Act.Abs)
                csp = pspv.tile([1, 512], dt.float32, tag="pv")
                nc.tensor.matmul(csp[:], ones_sb[:], ax2[:],
                                 start=True, stop=True)
                nc.scalar.copy(cs2_sb[:, 512 * t4:512 * (t4 + 1)], csp[:])
            nc.sync.dma_start(cs2_in[:], cs2_sb[:])
            nc.gpsimd.collective_compute(
                "AllReduce", Alu.add, replica_groups=RG,
                ins=[cs2_in[:]], outs=[cs2_out[:]])
            s2_sb = tiny.tile([1, T], dt.float32, tag="s1")
            nc.sync.dma_start(s2_sb[:], cs2_out[:])
            nc.vector.tensor_scalar(s2_sb[:], s2_sb[:], 1.0 / C, EPS, Alu.mult, Alu.add)
            r2_sb = tiny.tile([1, T], dt.float32, tag="r1")
            nc.vector.reciprocal_approx_fast(r2_sb[:], s2_sb[:])
            r2b_sb = tiny.tile([1, T], dt.bfloat16, tag="r1b")
            nc.scalar.copy(r2b_sb[:], r2_sb[:])
            inv2_sb = mid.tile([P, T], dt.bfloat16, tag="inv1")
            nc.gpsimd.partition_broadcast(inv2_sb[:], r2b_sb[:])
            h2sh_sb = mid.tile([P, T], dt.bfloat16, tag="attn")
            nc.vector.tensor_tensor(h2sh_sb[:], x2_sb[:], inv2_sb[:], Alu.mult)

            # ---------- AllGather(h2) ----------
            nc.sync.dma_start(h2_in[:], h2sh_sb[:])
            nc.gpsimd.collective_compute(
                "AllGather", Alu.bypass, replica_groups=RG,
                ins=[h2_in[:]], outs=[h2_out[:]])
            h2f_sb = big.tile([P, CS, T], dt.bfloat16, tag="big3")
            nc.sync.dma_start(h2f_sb[:], h2_out.ap().rearrange("(o p) t -> p o t", p=P))

            # ---------- P7: FFN1 + rational SwiGLU ----------
            hidT = mid.tile([P, 4, T], dt.bfloat16, tag="hid")
            for fc in range(4):
                for t4 in range(TT):
                    tsl = slice(512 * t4, 512 * (t4 + 1))
                    gp = psmm.tile([P, 512], dt.float32, tag="mm")
                    vp = psmm.tile([P, 512], dt.float32, tag="mm")
                    for o in range(CS):
                        nc.tensor.matmul(gp[:], wm_sb[:, o, 128 * fc:128 * (fc + 1)],
                                         h2f_sb[:, o, tsl], start=(o == 0), stop=(o == CS - 1))
                    for o in range(CS):
                        nc.tensor.matmul(vp[:], wm_sb[:, o, 512 + 128 * fc:512 + 128 * (fc + 1)],
                                         h2f_sb[:, o, tsl], start=(o == 0), stop=(o == CS - 1))
                    a2 = workA.tile([P, 512], dt.float32, tag="a")
                    nc.scalar.activation(a2[:], gp[:], Act.Abs)
                    b2 = workA.tile([P, 512], dt.float32, tag="b")
                    nc.vector.tensor_scalar(b2[:], a2[:], 1.0, None, Alu.add)
                    rr = workA.tile([P, 512], dt.float32, tag="r")
                    nc.vector.reciprocal_approx_fast(rr[:], b2[:])
                    nn = workA.tile([P, 512], dt.float32, tag="nn")
                    nc.vector.tensor_scalar(nn[:], gp[:], 0.0, 0.5, Alu.max, Alu.add)
                    nr = workB.tile([P, 512], dt.bfloat16, tag="sr")
                    nc.vector.tensor_tensor(nr[:], nn[:], rr[:], Alu.mult)
                    vs = workB.tile([P, 512], dt.bfloat16, tag="u2")
                    nc.scalar.copy(vs[:], vp[:])
                    gv = workB.tile([P, 512], dt.bfloat16, tag="p4")
                    nc.vector.tensor_tensor(gv[:], gp[:], vs[:], Alu.mult)
                    nc.vector.tensor_tensor(hidT[:, fc, tsl], nr[:], gv[:], Alu.mult)

            # ---------- P8: FFN2 (row-split) -> ReduceScatter ----------
            for jc in range(CS):
                for t4 in range(TT):
                    tsl = slice(512 * t4, 512 * (t4 + 1))
                    zp = psmm.tile([P, 512], dt.float32, tag="mm")
                    for o in range(4):
                        nc.tensor.matmul(zp[:], w3_sb[:, o, 128 * jc:128 * (jc + 1)],
                                         hidT[:, o, tsl], start=(o == 0), stop=(o == 3))
                    zs = workC.tile([P, 512], dt.bfloat16, tag="zs")
                    nc.scalar.copy(zs[:], zp[:])
                    nc.sync.dma_start(rs_in[P * jc:P * (jc + 1), tsl], zs[:])
            nc.gpsimd.collective_compute(
                "ReduceScatter", Alu.add, replica_groups=RG,
                ins=[rs_in[:]], outs=[rs_out[:]])

            # ---------- P9: final residual ----------
            rso_sb = mid.tile([P, T], dt.bfloat16, tag="rso")
            nc.sync.dma_start(rso_sb[:], rs_out[:])
            for t4 in range(TT):
                tsl = slice(512 * t4, 512 * (t4 + 1))
                of = workC.tile([P, 512], dt.float32, tag="xsl")
                nc.vector.tensor_tensor(of[:], rso_sb[:, tsl], x2_sb[:, tsl], Alu.add)
                nc.sync.dma_start(out[:, tsl], of[:])

    nc.compile()
    return nc


def _get_program():
    global _PROGRAM
    if _PROGRAM is None:
        _PROGRAM = _build_program()
    return _PROGRAM


def kernel(x, w_qkv, w_out, w_merged, w3, norm1_w, norm2_w):
    global LAST_RESULTS
    from concourse.bass_utils import run_bass_kernel_spmd

    nc = _get_program()
    in_maps = _prepare_in_maps(x, w_qkv, w_out, w_merged, w3, norm1_w, norm2_w)
    res = run_bass_kernel_spmd(nc, in_maps, core_ids=list(range(NCORES)),
                               trace=TRACE)
    LAST_RESULTS = res
    yT = np.concatenate([res.results[i]["out"] for i in range(NCORES)], axis=0)
    return np.ascontiguousarray(yT.T)[None].astype(np.float32)
